# revision 12
# baseline (speedup 1.0000x reference)
"""MixAttention Trainium2 kernel.

Reference computation (B=64, N=384, C=768, H=12, hd=64, Nt=128):
    qkv = x @ W_qkv + b_qkv -> q, k, v per head
    t2t: softmax(q[:, :128] @ k[:, :128].T * 1/8) @ v[:, :128]   (template)
    s2a: softmax(q[:, 128:] @ k.T * 1/8) @ v                     (search)
    out = concat @ W_proj + b_proj

Strategy: pure data-parallel over batch, 8 batches per core on 8 cores, no
collectives. All matmul contractions need channel-major (transposed)
operands; x is transposed once on the host (free vs. NEFF exec time). All
GEMMs run in bf16 (fp32 PSUM accumulation).

Per batch, three stages software-pipelined at emission so the PE stream of
one stage hides the ACT/DVE latency of another (HEAD b+1 | ATTN b | TAIL
b-1):
  HEAD: DMA xT slices; q/k projection (W stationary, xT moving) -> q and
        kT m-tiles [128 = 2 heads x 64, 384 tokens] (+bias via ACT); v
        projection (xT stationary, W_v moving) -> token-major v with a
        per-pair 160-col block layout [e(0:64) | ones(64:96) | o(96:160)]
        so both parities of a head pair get PV stationaries that place
        the head's output AND its softmax denominator at legal PSUM
        partitions (DVE PSUM reads require 32-aligned base partitions;
        partition_broadcast requires a base-0 destination).
  ATTN: per head pair: scores^T via K=64 row-tiled matmuls - head 2p's
        kT slice [64, keys] sits in array rows 0-63, head 2p+1's in rows
        64-127 (tile_position auto-derived from base partition), each
        streaming its own 64-partition half of the q m-tile. The two
        matmuls occupy disjoint row groups so the PE runs them
        concurrently - full-rate scores without zero-padding the K dim.
        Key chunks 1+2 share one [128, 512] PSUM bank so exp runs as one
        ACT op; chunk 0 spans all 384 queries. exp via ACT (softmax scale
        fused) -> E^T bf16. PV per head: even heads use stationary
        [keys, 65] = [dims | one] -> O^T psum rows 0-63 + denominator at
        row 64; odd heads use stationary [keys, 128] = cols base+32..160
        = [junk(32) | one | ones(31) | dims(64)] -> denominator at row 32
        (32-aligned), dims at rows 64-127, so the normalized output
        writes its xt2 c-chunk half directly (DVE is lane-locked; no
        partition shift needed). normalize: reciprocal (DVE, bf16) ->
        partition broadcast (GpSimd, base-0 dest; odd heads broadcast
        all 128 rows) -> multiply (DVE).
  TAIL: output projection (X^T chunks stationary, W_proj moving,
        accumulate over 6 c-chunks) + bias -> one [128, 768] tile per
        128-token chunk, DMA'd out with fully contiguous per-partition
        rows (3 DMAs per batch).
"""

import numpy as np

B, N, C = 64, 384, 768
H, HD = 12, 64
NT = 128          # template tokens (t_h * t_w * 2)
NCORES = 8
NB = B // NCORES  # batches per core
TOK = NB * N      # tokens per core

_PROGRAM = None

# v1 pair-block layout constants: per head pair a 160-col block
# [e_dims(0:64) | ones(64:96) | o_dims(96:160)]
VBLK = 160
VW = 6 * VBLK     # 960 cols total (6 pairs)


def _build_program(nbatch, e_bf16=True, loop_reps=1, bufs=None, ablate=(),
                   stagger=True):
    import contextlib
    import concourse.mybir as mybir
    import concourse.tile as tile
    from concourse import bacc

    f32 = mybir.dt.float32
    bf16 = mybir.dt.bfloat16
    e_dt = bf16 if e_bf16 else mybir.dt.float32r
    Act = mybir.ActivationFunctionType
    Alu = mybir.AluOpType

    bufs = dict(dict(x=2, qk=2, e=3, v=2, xa=2, xt2=2, o=3,
                     gemm=2, pss0=1, pss12=1, pv=2, r=4), **(bufs or {}))
    tok = nbatch * N
    nc = bacc.Bacc("TRN2", target_bir_lowering=False)

    xT = nc.dram_tensor("xT", [C, tok], bf16, kind="ExternalInput")
    wqkv = nc.dram_tensor("wqkv", [C, 3 * C], bf16, kind="ExternalInput")
    bqkv = nc.dram_tensor("bqkv", [3 * C], f32, kind="ExternalInput")
    wproj = nc.dram_tensor("wproj", [C, C], bf16, kind="ExternalInput")
    bproj = nc.dram_tensor("bproj", [C], f32, kind="ExternalInput")
    out = nc.dram_tensor("out", [tok, C], f32, kind="ExternalOutput")

    NCH = C // 128  # 6 c-chunks
    state = {}      # b -> dict of live tiles

    with tile.TileContext(nc) as tc:
        with (
            tc.tile_pool(name="wpool", bufs=1) as wpool,
            tc.tile_pool(name="xpool", bufs=bufs["x"]) as xpool,
            tc.tile_pool(name="qkpool", bufs=bufs["qk"]) as qkpool,
            tc.tile_pool(name="epool", bufs=bufs["e"]) as epool,
            tc.tile_pool(name="vpool", bufs=bufs["v"]) as vpool,
            tc.tile_pool(name="xt2pool", bufs=bufs["xt2"]) as xt2pool,
            tc.tile_pool(name="opool", bufs=bufs["o"]) as opool,
            tc.tile_pool(name="rpool", bufs=bufs["r"]) as rpool,
            tc.tile_pool(name="pspool", bufs=bufs["gemm"],
                         space="PSUM") as pspool,
            tc.tile_pool(name="pvpool", bufs=bufs["pv"],
                         space="PSUM") as pvpool,
        ):
            # ---- resident weights / constants ----
            w_qk, w_v, w_p = [], [], []
            for ci in range(NCH):
                t = wpool.tile([128, 2 * C], bf16, tag=f"wqk{ci}")
                nc.sync.dma_start(t[:], wqkv[ci * 128:(ci + 1) * 128, 0:2 * C])
                w_qk.append(t)
                t = wpool.tile([128, C], bf16, tag=f"wv{ci}")
                nc.sync.dma_start(t[:], wqkv[ci * 128:(ci + 1) * 128,
                                             2 * C:3 * C])
                w_v.append(t)
                t = wpool.tile([128, C], bf16, tag=f"wp{ci}")
                nc.sync.dma_start(t[:], wproj[ci * 128:(ci + 1) * 128, :])
                w_p.append(t)

            bqk = wpool.tile([128, 2 * C // 128], f32, tag="bqk")
            nc.sync.dma_start(
                bqk[:], bqkv[0:2 * C].rearrange("(m p) -> p m", p=128))
            bv_row = wpool.tile([1, C], f32, tag="bvrow")
            nc.sync.dma_start(bv_row[:],
                              bqkv[2 * C:3 * C].rearrange("(a c) -> a c", a=1))
            bv = wpool.tile([128, C], f32, tag="bv")
            nc.gpsimd.partition_broadcast(bv[:], bv_row[:])
            bp_row = wpool.tile([1, C], f32, tag="bprow")
            nc.sync.dma_start(bp_row[:],
                              bproj[:].rearrange("(a c) -> a c", a=1))
            bp = wpool.tile([128, C], f32, tag="bp")
            nc.gpsimd.partition_broadcast(bp[:], bp_row[:])

            def head(b):
                st = state[b] = {}
                xt = st["xt"] = []
                for ci in range(NCH):
                    t = xpool.tile([128, N], bf16, tag=f"xt{ci}",
                                   name=f"xt{ci}_{b}")
                    nc.sync.dma_start(
                        t[:], xT[ci * 128:(ci + 1) * 128, b * N:(b + 1) * N])
                    xt.append(t)

                qk = st["qk"] = []
                kt = st["kt"] = []
                for mt in range(2 * C // 128):  # q m-tiles 0-5, kT 6-11
                    ps = pspool.tile([128, N], f32, tag="gemm",
                                     name=f"psqk{mt}_{b}")
                    for ci in range(NCH):
                        nc.tensor.matmul(
                            ps[:], w_qk[ci][:, mt * 128:(mt + 1) * 128],
                            xt[ci][:], start=(ci == 0), stop=(ci == NCH - 1))
                    t = qkpool.tile([128, N], bf16, tag=f"qk{mt}",
                                    name=f"qk{mt}_{b}")
                    nc.scalar.activation(t[:], ps[:], Act.Identity,
                                         bias=bqk[:, mt:mt + 1], scale=1.0)
                    (qk if mt < 6 else kt).append(t)

                # v projection -> token-major pair-block layout (see module
                # docstring); cols 64:96 of each block are 1.0
                v1 = st["v1"] = []
                for tt in range(3):
                    t = vpool.tile([128, VW], e_dt, tag=f"v1{tt}",
                                   name=f"v1{tt}_{b}")
                    tv = t[:].rearrange("p (pr c) -> p pr c", pr=6)
                    nc.vector.memset(tv[:, :, 64:96], 1.0)
                    for half in range(2):
                        ps = pspool.tile([128, N], f32, tag="gemm",
                                         name=f"psv{tt}{half}_{b}")
                        for ci in range(NCH):
                            nc.tensor.matmul(
                                ps[:], xt[ci][:, tt * 128:(tt + 1) * 128],
                                w_v[ci][:, half * N:(half + 1) * N],
                                start=(ci == 0), stop=(ci == NCH - 1))
                        psv = ps[:].rearrange("p (pr h d) -> p pr h d",
                                              h=2, d=HD)
                        bvv = (bv[:, half * N:(half + 1) * N]
                               .rearrange("p (pr h d) -> p pr h d",
                                          h=2, d=HD))
                        prs = slice(3 * half, 3 * half + 3)
                        nc.vector.scalar_tensor_tensor(
                            out=tv[:, prs, 0:HD], in0=psv[:, :, 0, :],
                            scalar=1.0, in1=bvv[:, :, 0, :],
                            op0=Alu.mult, op1=Alu.add)
                        nc.vector.scalar_tensor_tensor(
                            out=tv[:, prs, 96:96 + HD], in0=psv[:, :, 1, :],
                            scalar=1.0, in1=bvv[:, :, 1, :],
                            op0=Alu.mult, op1=Alu.add)
                    v1.append(t)

            def attn_scores(b, hp):
                st = state[b]
                qt = st["qk"][hp]
                kt = st["kt"][hp]  # rows 0-63 head 2hp, rows 64-127 head 2hp+1
                # Emit the two parities' matmuls adjacently: they occupy
                # disjoint array row groups (base partition 0 vs 64) so the
                # PE executes them concurrently at the K=64 rate each.
                # Both parities' scores land in bank-pair PSUM tiles (2 banks
                # each): parity 0 in the first bank, parity 1 in the second,
                # so each softmax exp is ONE ACT op instead of two.
                ps0 = pspool.tile([128, 1024], f32, tag="pss0",
                                  bufs=bufs["pss0"], name=f"pss0_{hp}_{b}")
                for par in range(2):
                    lo = par * 64
                    nc.tensor.matmul(ps0[:, par * 512:par * 512 + N],
                                     kt[lo:lo + 64, 0:128],
                                     qt[lo:lo + 64, 0:N],
                                     start=True, stop=True)
                ps12 = pspool.tile([128, 1024], f32, tag="pss12",
                                   bufs=bufs["pss12"], name=f"pss12_{hp}_{b}")
                for jc in range(1, 3):
                    for par in range(2):
                        lo = par * 64
                        nc.tensor.matmul(
                            ps12[:, par * 512 + (jc - 1) * 256:
                                 par * 512 + jc * 256],
                            kt[lo:lo + 64, jc * 128:(jc + 1) * 128],
                            qt[lo:lo + 64, 128:N],
                            start=(jc == 1), stop=(jc == 2))
                e0 = epool.tile([128, 2 * N], e_dt, tag="e0",
                                name=f"e0_{hp}_{b}")
                nc.scalar.activation(
                    e0[:].rearrange("p (g c) -> p g c", g=2),
                    ps0[:].rearrange("p (g c) -> p g c", g=2)[:, :, 0:N],
                    Act.Exp, bias=0.0, scale=0.125)
                e12 = epool.tile([128, 1024], e_dt, tag="e12",
                                 name=f"e12_{hp}_{b}")
                nc.scalar.activation(e12[:], ps12[:],
                                     Act.Exp, bias=0.0, scale=0.125)
                for par in range(2):
                    st["e"][2 * hp + par] = (
                        e0[:, par * N:(par + 1) * N],
                        e12[:, par * 512:(par + 1) * 512])

            def attn_pv(b, h):
                st = state[b]
                v1 = st["v1"]
                xt2 = st["xt2"]
                e0, e12 = st["e"].pop(h)  # AP slices into shared pair tiles
                base = (h // 2) * VBLK
                if h % 2 == 0:
                    # stationary [keys, 65] = [dims | one]: O^T rows 0-63,
                    # denominator row 64
                    pv = pvpool.tile([HD + 1, N], f32, tag="pspv",
                                     name=f"pv{h}_{b}")
                    stat = [v1[tt][:, base:base + 65] for tt in range(3)]
                    den_row, dlo = HD, 0
                else:
                    # stationary [keys, 128] = cols base+32..base+160:
                    # denominator at row 32 (32-aligned for the DVE PSUM
                    # read), O^T rows 64-127 -> normalized output lands on
                    # partitions 64-127 directly
                    pv = pvpool.tile([128, N], f32, tag="pspv",
                                     name=f"pv{h}_{b}")
                    stat = [v1[tt][:, base + 32:base + 160]
                            for tt in range(3)]
                    den_row, dlo = 32, HD
                nc.tensor.matmul(pv[:, 0:128], stat[0],
                                 e0[:, 0:128], start=True, stop=True)
                nc.tensor.matmul(pv[:, 128:N], stat[0],
                                 e0[:, 128:N], start=True, stop=False)
                nc.tensor.matmul(pv[:, 128:N], stat[1],
                                 e12[:, 0:256], start=False, stop=False)
                nc.tensor.matmul(pv[:, 128:N], stat[2],
                                 e12[:, 256:512], start=False, stop=True)
                rr = rpool.tile([1, N], bf16, tag="rr", name=f"rr{h}_{b}")
                with nc.allow_low_precision(
                        reason="softmax denom recip in bf16: 2^-9 rel err"):
                    nc.vector.reciprocal(rr[:], pv[den_row:den_row + 1, :])
                brc = rpool.tile([128, N], bf16, tag="brc",
                                 name=f"brc{h}_{b}")
                # partition_broadcast requires a base-0 destination: even
                # heads fill rows 0-63, odd heads fill all 128 rows and the
                # multiply uses rows 64-127
                nc.gpsimd.partition_broadcast(brc[0:dlo + HD, :], rr[:])
                nc.vector.tensor_mul(xt2[h // 2][dlo:dlo + HD, :],
                                     pv[dlo:dlo + HD, :],
                                     brc[dlo:dlo + HD, :])

            def attn(b, lag=2):
                st = state[b]
                st["e"] = {}
                st["xt2"] = [
                    xt2pool.tile([128, N], bf16, tag=f"xt2{ci}",
                                 name=f"xt2{ci}_{b}")
                    for ci in range(NCH)]
                for hp in range(H // 2):
                    attn_scores(b, hp)
                    if hp >= lag:
                        attn_pv(b, 2 * (hp - lag))
                        attn_pv(b, 2 * (hp - lag) + 1)
                for hp in range(H // 2 - lag, H // 2):
                    attn_pv(b, 2 * hp)
                    attn_pv(b, 2 * hp + 1)

            def tail(b):
                st = state[b]
                xt2 = st["xt2"]
                for tt in range(3):
                    ot = opool.tile([128, C], f32, tag="osb",
                                    name=f"o{tt}_{b}")
                    for half in range(2):
                        ps = pspool.tile([128, N], f32, tag="gemm",
                                         name=f"pso{tt}{half}_{b}")
                        for ci in range(NCH):
                            nc.tensor.matmul(
                                ps[:], xt2[ci][:, tt * 128:(tt + 1) * 128],
                                w_p[ci][:, half * N:(half + 1) * N],
                                start=(ci == 0), stop=(ci == NCH - 1))
                        nc.vector.scalar_tensor_tensor(
                            out=ot[:, half * N:(half + 1) * N], in0=ps[:],
                            scalar=1.0, in1=bp[:, half * N:(half + 1) * N],
                            op0=Alu.mult, op1=Alu.add)
                    nc.sync.dma_start(
                        out[(b * 3 + tt) * 128:(b * 3 + tt + 1) * 128, :],
                        ot[:])
                del state[b]

            Eng = mybir.EngineType
            loop_cm = (tc.For_i(0, loop_reps, 1,
                                hint_engines=(Eng.PE, Eng.Activation,
                                              Eng.DVE, Eng.Pool, Eng.SP))
                       if loop_reps > 1 else contextlib.nullcontext())
            with loop_cm:
                if stagger:
                    for step in range(nbatch + 2):
                        if step < nbatch:
                            head(step)
                        if 0 <= step - 1 < nbatch:
                            attn(step - 1)
                        if 0 <= step - 2 < nbatch:
                            tail(step - 2)
                else:
                    for b in range(nbatch):
                        head(b)
                        attn(b)
                        tail(b)
    nc.compile()
    return nc


def _get_program():
    global _PROGRAM
    if _PROGRAM is None:
        _PROGRAM = _build_program(NB)
    return _PROGRAM


def make_in_maps(x, W_qkv, b_qkv, W_proj, b_proj):
    import ml_dtypes
    bf = ml_dtypes.bfloat16
    x = np.asarray(x, dtype=np.float32)
    W_qkv = np.asarray(W_qkv, dtype=np.float32).astype(bf)
    b_qkv = np.asarray(b_qkv, dtype=np.float32)
    W_proj = np.asarray(W_proj, dtype=np.float32).astype(bf)
    b_proj = np.asarray(b_proj, dtype=np.float32)
    in_maps = []
    for i in range(NCORES):
        xc = x[i * NB:(i + 1) * NB].reshape(TOK, C)
        in_maps.append({
            "xT": np.ascontiguousarray(xc.T).astype(bf),
            "wqkv": W_qkv, "bqkv": b_qkv,
            "wproj": W_proj, "bproj": b_proj,
        })
    return in_maps


def kernel(x, W_qkv, b_qkv, W_proj, b_proj, t_h, t_w, s_h, s_w):
    from concourse.bass_utils import run_bass_kernel_spmd

    x = np.asarray(x, dtype=np.float32)
    assert x.shape == (B, N, C)
    assert int(t_h) * int(t_w) * 2 == NT
    assert int(s_h) * int(s_w) == N - NT

    nc = _get_program()
    in_maps = make_in_maps(x, W_qkv, b_qkv, W_proj, b_proj)
    res = run_bass_kernel_spmd(nc, in_maps, core_ids=list(range(NCORES)))
    return np.concatenate(
        [r["out"].reshape(NB, N, C) for r in res.results], axis=0)


# revision 18
# speedup vs baseline: 1.2901x; 1.2901x over previous
"""MixAttention Trainium2 kernel.

Reference computation (B=64, N=384, C=768, H=12, hd=64, Nt=128):
    qkv = x @ W_qkv + b_qkv -> q, k, v per head
    t2t: softmax(q[:, :128] @ k[:, :128].T * 1/8) @ v[:, :128]   (template)
    s2a: softmax(q[:, 128:] @ k.T * 1/8) @ v                     (search)
    out = concat @ W_proj + b_proj

Strategy: pure data-parallel over batch, 8 batches per core on 8 cores, no
collectives. All matmul contractions need channel-major (transposed)
operands; x is transposed once on the host (free vs. NEFF exec time). All
GEMMs run in bf16 (fp32 PSUM accumulation) - fp8 is ruled out by the
error budget (e4m3 GEMM error ~4% > the 2e-2 gate).

Per batch, three stages software-pipelined at emission so the PE stream of
one stage hides the ACT/DVE latency of another (HEAD b+1 | ATTN b | TAIL
b-1):
  HEAD: DMA xT slices; q/k projection (W stationary, xT moving) -> q and
        kT m-tiles [128 = 2 heads x 64, 384 tokens], bias fused into the
        single ACT evacuation per m-tile; v projection (xT stationary,
        W_v moving) -> token-major v in one tile per batch with a
        per-pair 160-col block layout [e(0:64) | ones(64:96) | o(96:160)]
        so both parities of a head pair get PV stationaries that place
        the head's output AND its softmax denominator at legal PSUM
        partitions (DVE PSUM reads require 32-aligned base partitions;
        partition_broadcast requires a base-0 destination; both verified
        by HW probes - violating either wedges the device).
  ATTN: per head pair: scores^T via K=64 row-tiled matmuls - head 2p's
        kT slice [64, keys] sits in array rows 0-63, head 2p+1's in rows
        64-127 (tile_position auto-derived from base partition), each
        streaming its own 64-partition half of the q m-tile. The two
        matmuls occupy disjoint row groups so the PE runs them
        concurrently - full-rate scores without zero-padding the K dim.
        Both parities' scores land in bank-pair [128, 1024] PSUM tiles
        (parity 0 in the first bank, parity 1 in the second), so softmax
        exp is TWO ACT ops per pair (a strided one over the two jc0
        banks, a dense [128, 1024] one over the jc1+jc2 banks) instead
        of six. PV per head: even heads use stationary [keys, 65] =
        [dims | one] -> O^T psum rows 0-63 + denominator at row 64; odd
        heads use stationary [keys, 128] = cols base+32..160 ->
        denominator at row 32 (32-aligned), dims at rows 64-127, so the
        normalized output writes its xt2 c-chunk half directly (DVE is
        lane-locked; no partition shift needed). normalize: reciprocal
        (DVE, bf16) -> partition broadcast (GpSimd, base-0 dest; odd
        heads broadcast all 128 rows and the multiply uses rows 64-127)
        -> multiply (DVE).
  TAIL: output projection (X^T chunks stationary, W_proj moving,
        accumulate over 6 c-chunks) + bias -> one [128, 3*768] tile,
        DMA'd out per 128-token chunk with fully contiguous
        per-partition rows (3 DMAs per batch; the v1 kernel's 6 strided
        DMAs saturated the SP sequencer at 315 us/iteration - this
        restructure cut SP.SEQ to 64 us and was the single biggest win).
"""

import numpy as np

B, N, C = 64, 384, 768
H, HD = 12, 64
NT = 128          # template tokens (t_h * t_w * 2)
NCORES = 8
NB = B // NCORES  # batches per core
TOK = NB * N      # tokens per core

_PROGRAM = None

# v1 pair-block layout constants: per head pair a 160-col block
# [e_dims(0:64) | ones(64:96) | o_dims(96:160)]
VBLK = 160
VW = 6 * VBLK     # 960 cols total (6 pairs)


def _build_program(nbatch, e_bf16=True, loop_reps=1, bufs=None, ablate=(),
                   stagger=True, lag=1):
    import contextlib
    import concourse.mybir as mybir
    import concourse.tile as tile
    from concourse import bacc

    f32 = mybir.dt.float32
    bf16 = mybir.dt.bfloat16
    e_dt = bf16 if e_bf16 else mybir.dt.float32r
    Act = mybir.ActivationFunctionType
    Alu = mybir.AluOpType

    bufs = dict(dict(x=2, qk=2, e=2, v=2, xa=2, xt2=2, o=3,
                     gemm=2, pss0=1, pss12=1, pv=2, r=4), **(bufs or {}))
    tok = nbatch * N
    nc = bacc.Bacc("TRN2", target_bir_lowering=False)

    xT = nc.dram_tensor("xT", [C, tok], bf16, kind="ExternalInput")
    wqkv = nc.dram_tensor("wqkv", [C, 3 * C], bf16, kind="ExternalInput")
    bqkv = nc.dram_tensor("bqkv", [3 * C], f32, kind="ExternalInput")
    wproj = nc.dram_tensor("wproj", [C, C], bf16, kind="ExternalInput")
    bproj = nc.dram_tensor("bproj", [C], f32, kind="ExternalInput")
    out = nc.dram_tensor("out", [tok, C], f32, kind="ExternalOutput")

    NCH = C // 128  # 6 c-chunks
    state = {}      # b -> dict of live tiles

    with tile.TileContext(nc) as tc:
        with (
            tc.tile_pool(name="wpool", bufs=1) as wpool,
            tc.tile_pool(name="xpool", bufs=bufs["x"]) as xpool,
            tc.tile_pool(name="qkpool", bufs=bufs["qk"]) as qkpool,
            tc.tile_pool(name="epool", bufs=bufs["e"]) as epool,
            tc.tile_pool(name="vpool", bufs=bufs["v"]) as vpool,
            tc.tile_pool(name="xt2pool", bufs=bufs["xt2"]) as xt2pool,
            tc.tile_pool(name="opool", bufs=bufs["o"]) as opool,
            tc.tile_pool(name="rpool", bufs=bufs["r"]) as rpool,
            tc.tile_pool(name="pspool", bufs=bufs["gemm"],
                         space="PSUM") as pspool,
            tc.tile_pool(name="pvpool", bufs=bufs["pv"],
                         space="PSUM") as pvpool,
        ):
            # ---- resident weights / constants ----
            w_qk, w_v, w_p = [], [], []
            for ci in range(NCH):
                t = wpool.tile([128, 2 * C], bf16, tag=f"wqk{ci}")
                nc.sync.dma_start(t[:], wqkv[ci * 128:(ci + 1) * 128, 0:2 * C])
                w_qk.append(t)
                t = wpool.tile([128, C], bf16, tag=f"wv{ci}")
                nc.sync.dma_start(t[:], wqkv[ci * 128:(ci + 1) * 128,
                                             2 * C:3 * C])
                w_v.append(t)
                t = wpool.tile([128, C], bf16, tag=f"wp{ci}")
                nc.sync.dma_start(t[:], wproj[ci * 128:(ci + 1) * 128, :])
                w_p.append(t)

            bqk = wpool.tile([128, 2 * C // 128], f32, tag="bqk")
            nc.sync.dma_start(
                bqk[:], bqkv[0:2 * C].rearrange("(m p) -> p m", p=128))
            bv_row = wpool.tile([1, C], f32, tag="bvrow")
            nc.sync.dma_start(bv_row[:],
                              bqkv[2 * C:3 * C].rearrange("(a c) -> a c", a=1))
            bv = wpool.tile([128, C], f32, tag="bv")
            nc.gpsimd.partition_broadcast(bv[:], bv_row[:])
            bp_row = wpool.tile([1, C], f32, tag="bprow")
            nc.sync.dma_start(bp_row[:],
                              bproj[:].rearrange("(a c) -> a c", a=1))
            bp = wpool.tile([128, C], f32, tag="bp")
            nc.gpsimd.partition_broadcast(bp[:], bp_row[:])

            def head(b):
                st = state[b] = {}
                xt = st["xt"] = []
                for ci in range(NCH):
                    t = xpool.tile([128, N], bf16, tag=f"xt{ci}",
                                   name=f"xt{ci}_{b}")
                    nc.sync.dma_start(
                        t[:], xT[ci * 128:(ci + 1) * 128, b * N:(b + 1) * N])
                    xt.append(t)

                qk = st["qk"] = []
                kt = st["kt"] = []
                for mt in range(2 * C // 128):  # q m-tiles 0-5, kT 6-11
                    ps = pspool.tile([128, N], f32, tag="gemm",
                                     name=f"psqk{mt}_{b}")
                    for ci in range(NCH):
                        nc.tensor.matmul(
                            ps[:], w_qk[ci][:, mt * 128:(mt + 1) * 128],
                            xt[ci][:], start=(ci == 0), stop=(ci == NCH - 1))
                    t = qkpool.tile([128, N], bf16, tag=f"qk{mt}",
                                    name=f"qk{mt}_{b}")
                    nc.scalar.activation(t[:], ps[:], Act.Identity,
                                         bias=bqk[:, mt:mt + 1], scale=1.0)
                    (qk if mt < 6 else kt).append(t)

                # v projection -> token-major pair-block layout (see module
                # docstring); cols 64:96 of each block are 1.0
                vt = vpool.tile([128, 3 * VW], e_dt, tag="v1",
                                name=f"v1_{b}")
                st["v1"] = vt
                nc.vector.memset(
                    vt[:].rearrange("p (g c) -> p g c", g=18)[:, :, 64:96],
                    1.0)
                for tt in range(3):
                    tv = vt[:, tt * VW:(tt + 1) * VW].rearrange(
                        "p (pr c) -> p pr c", pr=6)
                    for half in range(2):
                        ps = pspool.tile([128, N], f32, tag="gemm",
                                         name=f"psv{tt}{half}_{b}")
                        for ci in range(NCH):
                            nc.tensor.matmul(
                                ps[:], xt[ci][:, tt * 128:(tt + 1) * 128],
                                w_v[ci][:, half * N:(half + 1) * N],
                                start=(ci == 0), stop=(ci == NCH - 1))
                        psv = ps[:].rearrange("p (pr h d) -> p pr h d",
                                              h=2, d=HD)
                        bvv = (bv[:, half * N:(half + 1) * N]
                               .rearrange("p (pr h d) -> p pr h d",
                                          h=2, d=HD))
                        prs = slice(3 * half, 3 * half + 3)
                        nc.vector.scalar_tensor_tensor(
                            out=tv[:, prs, 0:HD], in0=psv[:, :, 0, :],
                            scalar=1.0, in1=bvv[:, :, 0, :],
                            op0=Alu.mult, op1=Alu.add)
                        nc.vector.scalar_tensor_tensor(
                            out=tv[:, prs, 96:96 + HD], in0=psv[:, :, 1, :],
                            scalar=1.0, in1=bvv[:, :, 1, :],
                            op0=Alu.mult, op1=Alu.add)

            def attn_scores(b, hp):
                st = state[b]
                qt = st["qk"][hp]
                kt = st["kt"][hp]  # rows 0-63 head 2hp, rows 64-127 head 2hp+1
                # Emit the two parities' matmuls adjacently: they occupy
                # disjoint array row groups (base partition 0 vs 64) so the
                # PE executes them concurrently at the K=64 rate each.
                # Both parities' scores land in bank-pair PSUM tiles (2 banks
                # each): parity 0 in the first bank, parity 1 in the second,
                # so each softmax exp is ONE ACT op instead of two.
                ps0 = pspool.tile([128, 1024], f32, tag="pss0",
                                  bufs=bufs["pss0"], name=f"pss0_{hp}_{b}")
                for par in range(2):
                    lo = par * 64
                    nc.tensor.matmul(ps0[:, par * 512:par * 512 + N],
                                     kt[lo:lo + 64, 0:128],
                                     qt[lo:lo + 64, 0:N],
                                     start=True, stop=True)
                ps12 = pspool.tile([128, 1024], f32, tag="pss12",
                                   bufs=bufs["pss12"], name=f"pss12_{hp}_{b}")
                for jc in range(1, 3):
                    for par in range(2):
                        lo = par * 64
                        nc.tensor.matmul(
                            ps12[:, par * 512 + (jc - 1) * 256:
                                 par * 512 + jc * 256],
                            kt[lo:lo + 64, jc * 128:(jc + 1) * 128],
                            qt[lo:lo + 64, 128:N],
                            start=(jc == 1), stop=(jc == 2))
                e0 = epool.tile([128, 2 * N], e_dt, tag="e0",
                                name=f"e0_{hp}_{b}")
                nc.scalar.activation(
                    e0[:].rearrange("p (g c) -> p g c", g=2),
                    ps0[:].rearrange("p (g c) -> p g c", g=2)[:, :, 0:N],
                    Act.Exp, bias=0.0, scale=0.125)
                e12 = epool.tile([128, 1024], e_dt, tag="e12",
                                 name=f"e12_{hp}_{b}")
                nc.scalar.activation(e12[:], ps12[:],
                                     Act.Exp, bias=0.0, scale=0.125)
                for par in range(2):
                    st["e"][2 * hp + par] = (
                        e0[:, par * N:(par + 1) * N],
                        e12[:, par * 512:(par + 1) * 512])

            def attn_pv(b, h):
                st = state[b]
                v1 = st["v1"]
                xt2 = st["xt2"]
                e0, e12 = st["e"].pop(h)  # AP slices into shared pair tiles
                base = (h // 2) * VBLK
                if h % 2 == 0:
                    # stationary [keys, 65] = [dims | one]: O^T rows 0-63,
                    # denominator row 64
                    pv = pvpool.tile([HD + 1, N], f32, tag="pspv",
                                     name=f"pv{h}_{b}")
                    stat = [v1[:, tt * VW + base:tt * VW + base + 65]
                            for tt in range(3)]
                    den_row, dlo = HD, 0
                else:
                    # stationary [keys, 128] = cols base+32..base+160:
                    # denominator at row 32 (32-aligned for the DVE PSUM
                    # read), O^T rows 64-127 -> normalized output lands on
                    # partitions 64-127 directly
                    pv = pvpool.tile([128, N], f32, tag="pspv",
                                     name=f"pv{h}_{b}")
                    stat = [v1[:, tt * VW + base + 32:tt * VW + base + 160]
                            for tt in range(3)]
                    den_row, dlo = 32, HD
                nc.tensor.matmul(pv[:, 0:128], stat[0],
                                 e0[:, 0:128], start=True, stop=True)
                nc.tensor.matmul(pv[:, 128:N], stat[0],
                                 e0[:, 128:N], start=True, stop=False)
                nc.tensor.matmul(pv[:, 128:N], stat[1],
                                 e12[:, 0:256], start=False, stop=False)
                nc.tensor.matmul(pv[:, 128:N], stat[2],
                                 e12[:, 256:512], start=False, stop=True)
                rr = rpool.tile([1, N], bf16, tag="rr", name=f"rr{h}_{b}")
                with nc.allow_low_precision(
                        reason="softmax denom recip in bf16: 2^-9 rel err"):
                    nc.vector.reciprocal(rr[:], pv[den_row:den_row + 1, :])
                brc = rpool.tile([128, N], bf16, tag="brc",
                                 name=f"brc{h}_{b}")
                # partition_broadcast requires a base-0 destination: even
                # heads fill rows 0-63, odd heads fill all 128 rows and the
                # multiply uses rows 64-127
                nc.gpsimd.partition_broadcast(brc[0:dlo + HD, :], rr[:])
                nc.vector.tensor_mul(xt2[h // 2][dlo:dlo + HD, :],
                                     pv[dlo:dlo + HD, :],
                                     brc[dlo:dlo + HD, :])

            def attn(b, lag=lag):
                st = state[b]
                st["e"] = {}
                st["xt2"] = [
                    xt2pool.tile([128, N], bf16, tag=f"xt2{ci}",
                                 name=f"xt2{ci}_{b}")
                    for ci in range(NCH)]
                for hp in range(H // 2):
                    attn_scores(b, hp)
                    if hp >= lag:
                        attn_pv(b, 2 * (hp - lag))
                        attn_pv(b, 2 * (hp - lag) + 1)
                for hp in range(H // 2 - lag, H // 2):
                    attn_pv(b, 2 * hp)
                    attn_pv(b, 2 * hp + 1)

            def tail(b):
                st = state[b]
                xt2 = st["xt2"]
                ot = opool.tile([128, 3 * C], f32, tag="osb",
                                name=f"o_{b}")
                for tt in range(3):
                    for half in range(2):
                        ps = pspool.tile([128, N], f32, tag="gemm",
                                         name=f"pso{tt}{half}_{b}")
                        for ci in range(NCH):
                            nc.tensor.matmul(
                                ps[:], xt2[ci][:, tt * 128:(tt + 1) * 128],
                                w_p[ci][:, half * N:(half + 1) * N],
                                start=(ci == 0), stop=(ci == NCH - 1))
                        nc.vector.scalar_tensor_tensor(
                            out=ot[:, tt * C + half * N:
                                   tt * C + (half + 1) * N],
                            in0=ps[:], scalar=1.0,
                            in1=bp[:, half * N:(half + 1) * N],
                            op0=Alu.mult, op1=Alu.add)
                for tt in range(3):
                    nc.sync.dma_start(
                        out[(b * 3 + tt) * 128:(b * 3 + tt + 1) * 128, :],
                        ot[:, tt * C:(tt + 1) * C])
                del state[b]

            Eng = mybir.EngineType
            loop_cm = (tc.For_i(0, loop_reps, 1,
                                hint_engines=(Eng.PE, Eng.Activation,
                                              Eng.DVE, Eng.Pool, Eng.SP),
                                staggered_reset=bool(
                                    int(__import__("os").environ.get(
                                        "STAGRESET", "0")))
                                )
                       if loop_reps > 1 else contextlib.nullcontext())
            with loop_cm:
                if stagger:
                    for step in range(nbatch + 2):
                        if step < nbatch:
                            head(step)
                        if 0 <= step - 1 < nbatch:
                            attn(step - 1)
                        if 0 <= step - 2 < nbatch:
                            tail(step - 2)
                else:
                    for b in range(nbatch):
                        head(b)
                        attn(b)
                        tail(b)
    nc.compile()
    return nc


def _get_program():
    global _PROGRAM
    if _PROGRAM is None:
        _PROGRAM = _build_program(NB)
    return _PROGRAM


def make_in_maps(x, W_qkv, b_qkv, W_proj, b_proj):
    import ml_dtypes
    bf = ml_dtypes.bfloat16
    x = np.asarray(x, dtype=np.float32)
    W_qkv = np.asarray(W_qkv, dtype=np.float32).astype(bf)
    b_qkv = np.asarray(b_qkv, dtype=np.float32)
    W_proj = np.asarray(W_proj, dtype=np.float32).astype(bf)
    b_proj = np.asarray(b_proj, dtype=np.float32)
    in_maps = []
    for i in range(NCORES):
        xc = x[i * NB:(i + 1) * NB].reshape(TOK, C)
        in_maps.append({
            "xT": np.ascontiguousarray(xc.T).astype(bf),
            "wqkv": W_qkv, "bqkv": b_qkv,
            "wproj": W_proj, "bproj": b_proj,
        })
    return in_maps


def kernel(x, W_qkv, b_qkv, W_proj, b_proj, t_h, t_w, s_h, s_w):
    from concourse.bass_utils import run_bass_kernel_spmd

    x = np.asarray(x, dtype=np.float32)
    assert x.shape == (B, N, C)
    assert int(t_h) * int(t_w) * 2 == NT
    assert int(s_h) * int(s_w) == N - NT

    nc = _get_program()
    in_maps = make_in_maps(x, W_qkv, b_qkv, W_proj, b_proj)
    res = run_bass_kernel_spmd(nc, in_maps, core_ids=list(range(NCORES)))
    return np.concatenate(
        [r["out"].reshape(NB, N, C) for r in res.results], axis=0)


# revision 19
# speedup vs baseline: 1.3283x; 1.0296x over previous
"""MixAttention Trainium2 kernel.

Reference computation (B=64, N=384, C=768, H=12, hd=64, Nt=128):
    qkv = x @ W_qkv + b_qkv -> q, k, v per head
    t2t: softmax(q[:, :128] @ k[:, :128].T * 1/8) @ v[:, :128]   (template)
    s2a: softmax(q[:, 128:] @ k.T * 1/8) @ v                     (search)
    out = concat @ W_proj + b_proj

Strategy: pure data-parallel over batch, 8 batches per core on 8 cores, no
collectives. All matmul contractions need channel-major (transposed)
operands; x is transposed once on the host (free vs. NEFF exec time). All
GEMMs run in bf16 (fp32 PSUM accumulation) - fp8 is ruled out by the
error budget (e4m3 GEMM error ~4% > the 2e-2 gate).

Per batch, three stages software-pipelined at emission so the PE stream of
one stage hides the ACT/DVE latency of another (HEAD b+1 | ATTN b | TAIL
b-1):
  HEAD: DMA xT slices; q/k projection (W stationary, xT moving) -> q and
        kT m-tiles [128 = 2 heads x 64, 384 tokens], bias fused into the
        single ACT evacuation per m-tile; v projection (xT stationary,
        W_v moving) -> token-major v in one tile per batch with a
        per-pair 160-col block layout [e(0:64) | ones(64:96) | o(96:160)]
        so both parities of a head pair get PV stationaries that place
        the head's output AND its softmax denominator at legal PSUM
        partitions (DVE PSUM reads require 32-aligned base partitions;
        partition_broadcast requires a base-0 destination; both verified
        by HW probes - violating either wedges the device).
  ATTN: per head pair: scores^T via K=64 row-tiled matmuls - head 2p's
        kT slice [64, keys] sits in array rows 0-63, head 2p+1's in rows
        64-127 (tile_position auto-derived from base partition), each
        streaming its own 64-partition half of the q m-tile. The two
        matmuls occupy disjoint row groups so the PE runs them
        concurrently - full-rate scores without zero-padding the K dim.
        Both parities' scores land in bank-pair [128, 1024] PSUM tiles
        (parity 0 in the first bank, parity 1 in the second), so softmax
        exp is TWO ACT ops per pair (a strided one over the two jc0
        banks, a dense [128, 1024] one over the jc1+jc2 banks) instead
        of six. PV per head: even heads use stationary [keys, 65] =
        [dims | one] -> O^T psum rows 0-63 + denominator at row 64; odd
        heads use stationary [keys, 128] = cols base+32..160 ->
        denominator at row 32 (32-aligned), dims at rows 64-127, so the
        normalized output writes its xt2 c-chunk half directly (DVE is
        lane-locked; no partition shift needed). normalize: reciprocal
        (DVE, bf16) -> partition broadcast (GpSimd, base-0 dest; odd
        heads broadcast all 128 rows and the multiply uses rows 64-127)
        -> multiply (DVE).
  TAIL: output projection (X^T chunks stationary, W_proj moving,
        accumulate over 6 c-chunks) + bias -> one [128, 3*768] tile,
        DMA'd out per 128-token chunk with fully contiguous
        per-partition rows (3 DMAs per batch; the v1 kernel's 6 strided
        DMAs saturated the SP sequencer at 315 us/iteration - this
        restructure cut SP.SEQ to 64 us and was the single biggest win).
"""

import numpy as np

B, N, C = 64, 384, 768
H, HD = 12, 64
NT = 128          # template tokens (t_h * t_w * 2)
NCORES = 8
NB = B // NCORES  # batches per core
TOK = NB * N      # tokens per core

_PROGRAM = None

# v1 pair-block layout constants: per head pair a 160-col block
# [e_dims(0:64) | ones(64:96) | o_dims(96:160)]
VBLK = 160
VW = 6 * VBLK     # 960 cols total (6 pairs)


def _build_program(nbatch, e_bf16=True, loop_reps=1, bufs=None, ablate=(),
                   stagger=True, lag=1):
    import contextlib
    import concourse.mybir as mybir
    import concourse.tile as tile
    from concourse import bacc

    f32 = mybir.dt.float32
    bf16 = mybir.dt.bfloat16
    e_dt = bf16 if e_bf16 else mybir.dt.float32r
    Act = mybir.ActivationFunctionType
    Alu = mybir.AluOpType

    bufs = dict(dict(x=2, qk=2, e=2, v=2, xa=2, xt2=2, o=3,
                     gemm=2, pss0=1, pss12=1, pv=2, r=4), **(bufs or {}))
    tok = nbatch * N
    nc = bacc.Bacc("TRN2", target_bir_lowering=False)

    xT = nc.dram_tensor("xT", [C, tok], bf16, kind="ExternalInput")
    wqkv = nc.dram_tensor("wqkv", [C, 3 * C], bf16, kind="ExternalInput")
    bqkv = nc.dram_tensor("bqkv", [3 * C], f32, kind="ExternalInput")
    wproj = nc.dram_tensor("wproj", [C, C], bf16, kind="ExternalInput")
    bproj = nc.dram_tensor("bproj", [C], f32, kind="ExternalInput")
    out = nc.dram_tensor("out", [tok, C], f32, kind="ExternalOutput")

    NCH = C // 128  # 6 c-chunks
    state = {}      # b -> dict of live tiles

    with tile.TileContext(nc) as tc:
        with (
            tc.tile_pool(name="wpool", bufs=1) as wpool,
            tc.tile_pool(name="xpool", bufs=bufs["x"]) as xpool,
            tc.tile_pool(name="qkpool", bufs=bufs["qk"]) as qkpool,
            tc.tile_pool(name="epool", bufs=bufs["e"]) as epool,
            tc.tile_pool(name="vpool", bufs=bufs["v"]) as vpool,
            tc.tile_pool(name="xt2pool", bufs=bufs["xt2"]) as xt2pool,
            tc.tile_pool(name="opool", bufs=bufs["o"]) as opool,
            tc.tile_pool(name="rpool", bufs=bufs["r"]) as rpool,
            tc.tile_pool(name="pspool", bufs=bufs["gemm"],
                         space="PSUM") as pspool,
            tc.tile_pool(name="pvpool", bufs=bufs["pv"],
                         space="PSUM") as pvpool,
        ):
            # ---- resident weights / constants ----
            w_qk, w_v, w_p = [], [], []
            for ci in range(NCH):
                t = wpool.tile([128, 2 * C], bf16, tag=f"wqk{ci}")
                nc.sync.dma_start(t[:], wqkv[ci * 128:(ci + 1) * 128, 0:2 * C])
                w_qk.append(t)
                t = wpool.tile([128, C], bf16, tag=f"wv{ci}")
                nc.sync.dma_start(t[:], wqkv[ci * 128:(ci + 1) * 128,
                                             2 * C:3 * C])
                w_v.append(t)
                t = wpool.tile([128, C], bf16, tag=f"wp{ci}")
                nc.sync.dma_start(t[:], wproj[ci * 128:(ci + 1) * 128, :])
                w_p.append(t)

            bqk = wpool.tile([128, 2 * C // 128], f32, tag="bqk")
            nc.sync.dma_start(
                bqk[:], bqkv[0:2 * C].rearrange("(m p) -> p m", p=128))
            bv_row = wpool.tile([1, C], f32, tag="bvrow")
            nc.sync.dma_start(bv_row[:],
                              bqkv[2 * C:3 * C].rearrange("(a c) -> a c", a=1))
            bv = wpool.tile([128, C], f32, tag="bv")
            nc.gpsimd.partition_broadcast(bv[:], bv_row[:])
            bp_row = wpool.tile([1, C], f32, tag="bprow")
            nc.sync.dma_start(bp_row[:],
                              bproj[:].rearrange("(a c) -> a c", a=1))
            bp = wpool.tile([128, C], f32, tag="bp")
            nc.gpsimd.partition_broadcast(bp[:], bp_row[:])

            def head(b):
                st = state[b] = {}
                xt = st["xt"] = []
                for ci in range(NCH):
                    t = xpool.tile([128, N], bf16, tag=f"xt{ci}",
                                   name=f"xt{ci}_{b}")
                    nc.sync.dma_start(
                        t[:], xT[ci * 128:(ci + 1) * 128, b * N:(b + 1) * N])
                    xt.append(t)

                qk = st["qk"] = []
                kt = st["kt"] = []
                for mt in range(2 * C // 128):  # q m-tiles 0-5, kT 6-11
                    ps = pspool.tile([128, N], f32, tag="gemm",
                                     name=f"psqk{mt}_{b}")
                    for ci in range(NCH):
                        nc.tensor.matmul(
                            ps[:], w_qk[ci][:, mt * 128:(mt + 1) * 128],
                            xt[ci][:], start=(ci == 0), stop=(ci == NCH - 1))
                    t = qkpool.tile([128, N], bf16, tag=f"qk{mt}",
                                    name=f"qk{mt}_{b}")
                    nc.scalar.activation(t[:], ps[:], Act.Identity,
                                         bias=bqk[:, mt:mt + 1], scale=1.0)
                    (qk if mt < 6 else kt).append(t)

                # v projection -> token-major pair-block layout (see module
                # docstring); cols 64:96 of each block are 1.0
                vt = vpool.tile([128, 3 * VW], e_dt, tag="v1",
                                name=f"v1_{b}")
                st["v1"] = vt
                nc.vector.memset(
                    vt[:].rearrange("p (g c) -> p g c", g=18)[:, :, 64:96],
                    1.0)
                for tt in range(3):
                    tv = vt[:, tt * VW:(tt + 1) * VW].rearrange(
                        "p (pr c) -> p pr c", pr=6)
                    for half in range(2):
                        ps = pspool.tile([128, N], f32, tag="gemm",
                                         name=f"psv{tt}{half}_{b}")
                        for ci in range(NCH):
                            nc.tensor.matmul(
                                ps[:], xt[ci][:, tt * 128:(tt + 1) * 128],
                                w_v[ci][:, half * N:(half + 1) * N],
                                start=(ci == 0), stop=(ci == NCH - 1))
                        psv = ps[:].rearrange("p (pr h d) -> p pr h d",
                                              h=2, d=HD)
                        bvv = (bv[:, half * N:(half + 1) * N]
                               .rearrange("p (pr h d) -> p pr h d",
                                          h=2, d=HD))
                        prs = slice(3 * half, 3 * half + 3)
                        nc.vector.scalar_tensor_tensor(
                            out=tv[:, prs, 0:HD], in0=psv[:, :, 0, :],
                            scalar=1.0, in1=bvv[:, :, 0, :],
                            op0=Alu.mult, op1=Alu.add)
                        nc.vector.scalar_tensor_tensor(
                            out=tv[:, prs, 96:96 + HD], in0=psv[:, :, 1, :],
                            scalar=1.0, in1=bvv[:, :, 1, :],
                            op0=Alu.mult, op1=Alu.add)

            def attn_scores(b, hp):
                st = state[b]
                qt = st["qk"][hp]
                kt = st["kt"][hp]  # rows 0-63 head 2hp, rows 64-127 head 2hp+1
                # Emit the two parities' matmuls adjacently: they occupy
                # disjoint array row groups (base partition 0 vs 64) so the
                # PE executes them concurrently at the K=64 rate each.
                # Both parities' scores land in bank-pair PSUM tiles (2 banks
                # each): parity 0 in the first bank, parity 1 in the second,
                # so each softmax exp is ONE ACT op instead of two.
                ps0 = pspool.tile([128, 1024], f32, tag="pss0",
                                  bufs=bufs["pss0"], name=f"pss0_{hp}_{b}")
                for par in range(2):
                    lo = par * 64
                    nc.tensor.matmul(ps0[:, par * 512:par * 512 + N],
                                     kt[lo:lo + 64, 0:128],
                                     qt[lo:lo + 64, 0:N],
                                     start=True, stop=True)
                ps12 = pspool.tile([128, 1024], f32, tag="pss12",
                                   bufs=bufs["pss12"], name=f"pss12_{hp}_{b}")
                for jc in range(1, 3):
                    for par in range(2):
                        lo = par * 64
                        nc.tensor.matmul(
                            ps12[:, par * 512 + (jc - 1) * 256:
                                 par * 512 + jc * 256],
                            kt[lo:lo + 64, jc * 128:(jc + 1) * 128],
                            qt[lo:lo + 64, 128:N],
                            start=(jc == 1), stop=(jc == 2))
                e0 = epool.tile([128, 2 * N], e_dt, tag="e0",
                                name=f"e0_{hp}_{b}")
                nc.scalar.activation(
                    e0[:].rearrange("p (g c) -> p g c", g=2),
                    ps0[:].rearrange("p (g c) -> p g c", g=2)[:, :, 0:N],
                    Act.Exp, bias=0.0, scale=0.125)
                e12 = epool.tile([128, 1024], e_dt, tag="e12",
                                 name=f"e12_{hp}_{b}")
                nc.scalar.activation(e12[:], ps12[:],
                                     Act.Exp, bias=0.0, scale=0.125)
                for par in range(2):
                    st["e"][2 * hp + par] = (
                        e0[:, par * N:(par + 1) * N],
                        e12[:, par * 512:(par + 1) * 512])

            def attn_pv(b, h):
                st = state[b]
                v1 = st["v1"]
                xt2 = st["xt2"]
                e0, e12 = st["e"].pop(h)  # AP slices into shared pair tiles
                base = (h // 2) * VBLK
                if h % 2 == 0:
                    # stationary [keys, 65] = [dims | one]: O^T rows 0-63,
                    # denominator row 64
                    pv = pvpool.tile([HD + 1, N], f32, tag="pspv",
                                     name=f"pv{h}_{b}")
                    stat = [v1[:, tt * VW + base:tt * VW + base + 65]
                            for tt in range(3)]
                    den_row, dlo = HD, 0
                else:
                    # stationary [keys, 128] = cols base+32..base+160:
                    # denominator at row 32 (32-aligned for the DVE PSUM
                    # read), O^T rows 64-127 -> normalized output lands on
                    # partitions 64-127 directly
                    pv = pvpool.tile([128, N], f32, tag="pspv",
                                     name=f"pv{h}_{b}")
                    stat = [v1[:, tt * VW + base + 32:tt * VW + base + 160]
                            for tt in range(3)]
                    den_row, dlo = 32, HD
                nc.tensor.matmul(pv[:, 0:128], stat[0],
                                 e0[:, 0:128], start=True, stop=True)
                nc.tensor.matmul(pv[:, 128:N], stat[0],
                                 e0[:, 128:N], start=True, stop=False)
                nc.tensor.matmul(pv[:, 128:N], stat[1],
                                 e12[:, 0:256], start=False, stop=False)
                nc.tensor.matmul(pv[:, 128:N], stat[2],
                                 e12[:, 256:512], start=False, stop=True)
                # evacuate dims to SBUF via ACT immediately so the PSUM
                # bank frees before the normalize chain completes (pv bufs=2
                # is the binding PSUM constraint); the multiply then runs
                # SBUF x SBUF in bf16 at the DVE 2x tier
                xa = rpool.tile([128, N], bf16, tag="xa", name=f"xa{h}_{b}")
                nc.scalar.activation(xa[dlo:dlo + HD, :], pv[dlo:dlo + HD, :],
                                     Act.Identity, bias=0.0, scale=1.0)
                rr = rpool.tile([1, N], bf16, tag="rr", name=f"rr{h}_{b}")
                with nc.allow_low_precision(
                        reason="softmax denom recip in bf16: 2^-9 rel err"):
                    nc.vector.reciprocal(rr[:], pv[den_row:den_row + 1, :])
                brc = rpool.tile([128, N], bf16, tag="brc",
                                 name=f"brc{h}_{b}")
                # partition_broadcast requires a base-0 destination: even
                # heads fill rows 0-63, odd heads fill all 128 rows and the
                # multiply uses rows 64-127
                nc.gpsimd.partition_broadcast(brc[0:dlo + HD, :], rr[:])
                nc.vector.tensor_mul(xt2[h // 2][dlo:dlo + HD, :],
                                     xa[dlo:dlo + HD, :],
                                     brc[dlo:dlo + HD, :])

            def attn(b, lag=lag):
                st = state[b]
                st["e"] = {}
                st["xt2"] = [
                    xt2pool.tile([128, N], bf16, tag=f"xt2{ci}",
                                 name=f"xt2{ci}_{b}")
                    for ci in range(NCH)]
                for hp in range(H // 2):
                    attn_scores(b, hp)
                    if hp >= lag:
                        attn_pv(b, 2 * (hp - lag))
                        attn_pv(b, 2 * (hp - lag) + 1)
                for hp in range(H // 2 - lag, H // 2):
                    attn_pv(b, 2 * hp)
                    attn_pv(b, 2 * hp + 1)

            def tail(b):
                st = state[b]
                xt2 = st["xt2"]
                ot = opool.tile([128, 3 * C], f32, tag="osb",
                                name=f"o_{b}")
                for tt in range(3):
                    for half in range(2):
                        ps = pspool.tile([128, N], f32, tag="gemm",
                                         name=f"pso{tt}{half}_{b}")
                        for ci in range(NCH):
                            nc.tensor.matmul(
                                ps[:], xt2[ci][:, tt * 128:(tt + 1) * 128],
                                w_p[ci][:, half * N:(half + 1) * N],
                                start=(ci == 0), stop=(ci == NCH - 1))
                        nc.vector.scalar_tensor_tensor(
                            out=ot[:, tt * C + half * N:
                                   tt * C + (half + 1) * N],
                            in0=ps[:], scalar=1.0,
                            in1=bp[:, half * N:(half + 1) * N],
                            op0=Alu.mult, op1=Alu.add)
                for tt in range(3):
                    nc.sync.dma_start(
                        out[(b * 3 + tt) * 128:(b * 3 + tt + 1) * 128, :],
                        ot[:, tt * C:(tt + 1) * C])
                del state[b]

            Eng = mybir.EngineType
            loop_cm = (tc.For_i(0, loop_reps, 1,
                                hint_engines=(Eng.PE, Eng.Activation,
                                              Eng.DVE, Eng.Pool, Eng.SP),
                                staggered_reset=bool(
                                    int(__import__("os").environ.get(
                                        "STAGRESET", "0")))
                                )
                       if loop_reps > 1 else contextlib.nullcontext())
            with loop_cm:
                if stagger:
                    for step in range(nbatch + 2):
                        if step < nbatch:
                            head(step)
                        if 0 <= step - 1 < nbatch:
                            attn(step - 1)
                        if 0 <= step - 2 < nbatch:
                            tail(step - 2)
                else:
                    for b in range(nbatch):
                        head(b)
                        attn(b)
                        tail(b)
    nc.compile()
    return nc


def _get_program():
    global _PROGRAM
    if _PROGRAM is None:
        _PROGRAM = _build_program(NB)
    return _PROGRAM


def make_in_maps(x, W_qkv, b_qkv, W_proj, b_proj):
    import ml_dtypes
    bf = ml_dtypes.bfloat16
    x = np.asarray(x, dtype=np.float32)
    W_qkv = np.asarray(W_qkv, dtype=np.float32).astype(bf)
    b_qkv = np.asarray(b_qkv, dtype=np.float32)
    W_proj = np.asarray(W_proj, dtype=np.float32).astype(bf)
    b_proj = np.asarray(b_proj, dtype=np.float32)
    in_maps = []
    for i in range(NCORES):
        xc = x[i * NB:(i + 1) * NB].reshape(TOK, C)
        in_maps.append({
            "xT": np.ascontiguousarray(xc.T).astype(bf),
            "wqkv": W_qkv, "bqkv": b_qkv,
            "wproj": W_proj, "bproj": b_proj,
        })
    return in_maps


def kernel(x, W_qkv, b_qkv, W_proj, b_proj, t_h, t_w, s_h, s_w):
    from concourse.bass_utils import run_bass_kernel_spmd

    x = np.asarray(x, dtype=np.float32)
    assert x.shape == (B, N, C)
    assert int(t_h) * int(t_w) * 2 == NT
    assert int(s_h) * int(s_w) == N - NT

    nc = _get_program()
    in_maps = make_in_maps(x, W_qkv, b_qkv, W_proj, b_proj)
    res = run_bass_kernel_spmd(nc, in_maps, core_ids=list(range(NCORES)))
    return np.concatenate(
        [r["out"].reshape(NB, N, C) for r in res.results], axis=0)
